# revision 24
# baseline (speedup 1.0000x reference)
"""BinaryXnorExceptOutliersLinear on 8 Trainium2 NeuronCores.

Reference math:
    mask, bscale from global kth-value quantiles of w
    w_q  = per-row asymmetric 8-bit fake quant of w
    w_sim = mask ? w_q : sign(w_q)*bscale
    out  = x @ w_sim.T + bias

Strategy: the weight transform is data-independent of x, so it is done on
the host (numpy, f32, op-for-op like the reference). The device kernel is
a pure streaming GEMM over an fp8(e3m4) encoding of w_sim: per out-row o,
codes = w_sim[o,:]/s_o with s_o = bscale/nb_o and nb_o the largest
e3m4-exact value such that max|codes| <= 15.5. Non-outliers (+-bscale,
95% of weights) encode EXACTLY as +-nb_o; only outliers carry e3m4
rounding (~3%), giving ~7e-3 output rel err. Each core streams its
pre-transposed [8192 in, 1024 out] fp8 shard (8 MiB) as the PE moving
operand against stationary f16 x chunks, accumulating in two 512-wide
PSUM banks; the host applies s_o per column, adds bias, and concatenates
the 8 shards.
"""
import sys

sys.path.insert(0, "/opt/trn_rl_repo")

import numpy as np
import ml_dtypes
from contextlib import ExitStack

import bass_rust
import concourse.bass as bass
import concourse.mybir as mybir
import concourse.tile as tile
from concourse.bass_utils import run_bass_kernel_spmd

# ---------------------------------------------------------------------------
OUT_F = 8192
IN_F = 8192
BATCH = 32
N_CORES = 8
ROWS_PER_CORE = OUT_F // N_CORES      # 1024
P = 128
CH = IN_F // P                         # 64 contraction chunks
OUTLIER_FRACTION = 0.05
F8MAX = 15.5                           # e3m4 max finite

f32 = mybir.dt.float32
f16 = mybir.dt.float16
f8 = mybir.dt.float8e3

# ---------------------------------------------------------------------------
# walrus compatibility


def _prepare_for_walrus(nc):
    mybir.codegen_inst_isa_subclasses(nc)
    ctr = 0
    for bb in nc.main_func.blocks:
        new = []
        for inst in bb.instructions:
            si = inst.sync_info
            if si is not None and len(si.on_wait) > 1:
                waits = list(si.on_wait)
                for w in waits[:-1]:
                    nop = bass_rust.InstNoOp(
                        name=f"I-wsplit-{ctr}", engine=inst.engine
                    )
                    ctr += 1
                    nop.sync_info = mybir.SyncInfo(on_wait=[w], on_update=[])
                    try:
                        nc.register_instruction(nop, overwrite=True)
                    except Exception:
                        pass
                    new.append(nop)
                si.on_wait = [waits[-1]]
            new.append(inst)
        bb.instructions = new
    return nc


# ---------------------------------------------------------------------------
# device program: psum[32, 1024] = x16[32, 8192] @ codes[8192, 1024]

# staged weight-stream schedule (units of 128-row contraction chunks):
# small chunks first for an early PE start, 1 MiB chunks after for DMA
# efficiency; big pool fully resident so DMA never stalls on reuse
SCHED = [2, 2] + [4] * 15
assert sum(SCHED) == CH


def _build_nc():
    nc = bass.Bass()
    # host layout: wP[p, c, o] = codesT[c*128 + p, o] -> contiguous lines
    wP = nc.dram_tensor("wP", [P, CH * ROWS_PER_CORE], f8,
                        kind="ExternalInput")
    xS = nc.dram_tensor("xS", [P, CH * BATCH], f16, kind="ExternalInput")
    y = nc.dram_tensor("y", [BATCH, ROWS_PER_CORE], f32, kind="ExternalOutput")

    A = mybir.AluOpType

    with tile.TileContext(nc) as tc, ExitStack() as ctx:
        const_pool = ctx.enter_context(tc.tile_pool(name="const", bufs=1))
        pool_m2 = ctx.enter_context(tc.tile_pool(name="wm2", bufs=2))
        pool_m4 = ctx.enter_context(tc.tile_pool(name="wm4", bufs=15))
        pool_l = ctx.enter_context(tc.tile_pool(name="wl", bufs=6))
        psum = ctx.enter_context(tc.tile_pool(name="psum", bufs=1, space="PSUM"))

        # x, host-laid-out as [p, c, b]; two transfers so the first chunk's
        # matmuls gate only on the first half
        xt = const_pool.tile([P, CH, BATCH], f16)
        HX = CH // 2 * BATCH
        nc.sync.dma_start(xt[:, 0:CH // 2, :], xS[:, 0:HX])
        nc.scalar.dma_start(xt[:, CH // 2:CH, :], xS[:, HX:2 * HX])

        ps0 = psum.tile([BATCH, 512], f32, tag="ps0")
        ps1 = psum.tile([BATCH, 512], f32, tag="ps1")

        c = 0
        for k, w in enumerate(SCHED):
            pool = {2: pool_m2, 4: pool_m4, 8: pool_l}[w]
            wt = pool.tile([P, w, ROWS_PER_CORE], f8)
            eng = nc.sync if k % 2 == 0 else nc.scalar
            eng.dma_start(
                wt[:],
                wP[:, c * ROWS_PER_CORE:(c + w) * ROWS_PER_CORE],
            )
            last = (k == len(SCHED) - 1)
            if last:
                # finish ps0 first so its copy overlaps ps1's last matmuls
                for j in range(w):
                    cc = c + j
                    nc.tensor.matmul(ps0[:], xt[:, cc, :], wt[:, j, 0:512],
                                     start=(cc == 0), stop=(cc == CH - 1))
                for j in range(w):
                    cc = c + j
                    nc.tensor.matmul(ps1[:], xt[:, cc, :], wt[:, j, 512:1024],
                                     start=(cc == 0), stop=(cc == CH - 1))
                c += w
            else:
                for j in range(w):
                    st, sp = (c == 0), False
                    nc.tensor.matmul(ps0[:], xt[:, c, :], wt[:, j, 0:512],
                                     start=st, stop=sp)
                    nc.tensor.matmul(ps1[:], xt[:, c, :], wt[:, j, 512:1024],
                                     start=st, stop=sp)
                    c += 1

        opool = ctx.enter_context(tc.tile_pool(name="o", bufs=1))
        ot = opool.tile([BATCH, ROWS_PER_CORE], f32)
        nc.scalar.copy(ot[:, 0:512], ps0[:])
        nc.sync.dma_start(y[:, 0:512], ot[:, 0:512])
        nc.vector.tensor_scalar(ot[:, 512:768], ps1[:, 0:256], 0.0, None,
                                A.add)
        nc.scalar.copy(ot[:, 768:1024], ps1[:, 256:512])
        nc.scalar.dma_start(y[:, 512:1024], ot[:, 512:1024])

    _prepare_for_walrus(nc)
    return nc


_NC_CACHE = None


def _get_nc():
    global _NC_CACHE
    if _NC_CACHE is None:
        _NC_CACHE = _build_nc()
    return _NC_CACHE


# ---------------------------------------------------------------------------
# host precompute: reference weight transform + e3m4 encoding


def _host_wsim(weight):
    w = np.ascontiguousarray(weight, dtype=np.float32)
    n = w.size
    k_lo = int(n * OUTLIER_FRACTION / 2)
    k_hi = int(n * (1.0 - OUTLIER_FRACTION / 2))
    part = np.partition(w.reshape(-1), [k_lo - 1, k_hi - 1])
    lo = np.float32(part[k_lo - 1])
    hi = np.float32(part[k_hi - 1])
    mask = (w < lo) | (w > hi)
    keep = ~mask
    bscale = np.float32(
        np.sum(np.abs(w) * keep, dtype=np.float32)
        / np.sum(keep, dtype=np.float32)
    )
    # per-row asymmetric 8-bit fake quant, f32 op-for-op like the reference
    w_min = w.min(1, keepdims=True).astype(np.float32)
    w_max = w.max(1, keepdims=True).astype(np.float32)
    rng = (w_max - w_min).astype(np.float32)
    zp = np.round(w_min - np.float32(128.0) * rng / np.float32(255.0)).astype(
        np.float32)
    q = (w - zp).astype(np.float32)
    q = (q * np.float32(255.0)).astype(np.float32)
    q = (q / rng).astype(np.float32)
    q = np.clip(np.round(q), np.float32(0.0), np.float32(255.0)).astype(
        np.float32)
    w_q = (q * (rng / np.float32(255.0)) + zp).astype(np.float32)
    w_sim = np.where(mask, w_q, np.sign(w_q) * bscale).astype(np.float32)
    return w_sim, bscale


def _snap_down_e3m4(v):
    """Largest e3m4-exact value <= v (v positive)."""
    c = v.astype(ml_dtypes.float8_e3m4)
    cf = c.astype(np.float32)
    bits = c.view(np.uint8)
    bits = np.where(cf > v, bits - 1, bits)
    return bits.view(ml_dtypes.float8_e3m4).astype(np.float32)


def _encode_e3m4(w_sim, bscale):
    M = np.abs(w_sim).max(1)
    nb_t = (np.float32(F8MAX) * bscale / M * np.float32(0.999)).astype(
        np.float32)
    nb = _snap_down_e3m4(nb_t)
    s = (bscale / nb).astype(np.float32)
    codes_f = np.clip(w_sim / s[:, None], -F8MAX, F8MAX)
    codes = codes_f.astype(ml_dtypes.float8_e3m4)
    return codes, s


def _run(inputs, trace=False):
    x, weight, bias = inputs["x"], inputs["weight"], inputs["bias"]
    w_sim, bscale = _host_wsim(weight)
    codes, s = _encode_e3m4(w_sim, bscale)

    x2 = np.ascontiguousarray(x, dtype=np.float32).reshape(BATCH, IN_F)
    # [p, c, b] layout: in-feature i = c*128 + p
    xS = np.ascontiguousarray(
        x2.T.reshape(CH, P, BATCH).transpose(1, 0, 2).reshape(P, CH * BATCH)
    ).astype(np.float16)

    nc = _get_nc()
    in_maps = []
    for c in range(N_CORES):
        sl = slice(c * ROWS_PER_CORE, (c + 1) * ROWS_PER_CORE)
        # [in, out] -> [p, c, o] with in = c*128 + p
        cT = codes[sl].T.reshape(CH, P, ROWS_PER_CORE)
        wPc = np.ascontiguousarray(cT.transpose(1, 0, 2)).reshape(
            P, CH * ROWS_PER_CORE)
        in_maps.append({
            "wP": wPc,
            "xS": xS,
        })
    res = run_bass_kernel_spmd(
        nc, in_maps, core_ids=list(range(N_CORES)), trace=trace
    )
    ys = np.concatenate([r["y"] for r in res.results], axis=1)  # [32, 8192]
    out = (ys * s[None, :] + np.asarray(bias, np.float32)[None, :]).reshape(
        BATCH, 1, OUT_F).astype(np.float32)
    return out, res


def kernel(**inputs):
    out, _ = _run(inputs, trace=False)
    return out


# revision 26
# speedup vs baseline: 1.0034x; 1.0034x over previous
"""BinaryXnorExceptOutliersLinear on 8 Trainium2 NeuronCores.

Reference math:
    mask, bscale from global kth-value quantiles of w
    w_q  = per-row asymmetric 8-bit fake quant of w
    w_sim = mask ? w_q : sign(w_q)*bscale
    out  = x @ w_sim.T + bias

Strategy: the weight transform is data-independent of x, so it is done on
the host (numpy, f32, op-for-op like the reference). The device kernel is
a pure streaming GEMM over an fp8(e3m4) encoding of w_sim: per out-row o,
codes = w_sim[o,:]/s_o with s_o = bscale/nb_o and nb_o the largest
e3m4-exact value such that max|codes| <= 15.5. Non-outliers (+-bscale,
95% of weights) encode EXACTLY as +-nb_o; only outliers carry e3m4
rounding (~3%), giving ~7e-3 output rel err. Each core streams its
pre-transposed [8192 in, 1024 out] fp8 shard (8 MiB) as the PE moving
operand against stationary f16 x chunks, accumulating in two 512-wide
PSUM banks; the host applies s_o per column, adds bias, and concatenates
the 8 shards.
"""
import sys

sys.path.insert(0, "/opt/trn_rl_repo")

import numpy as np
import ml_dtypes
from contextlib import ExitStack

import bass_rust
import concourse.bass as bass
import concourse.mybir as mybir
import concourse.tile as tile
from concourse.bass_utils import run_bass_kernel_spmd

# ---------------------------------------------------------------------------
OUT_F = 8192
IN_F = 8192
BATCH = 32
N_CORES = 8
ROWS_PER_CORE = OUT_F // N_CORES      # 1024
P = 128
CH = IN_F // P                         # 64 contraction chunks
OUTLIER_FRACTION = 0.05
F8MAX = 15.5                           # e3m4 max finite

f32 = mybir.dt.float32
f16 = mybir.dt.float16
f8 = mybir.dt.float8e3

# ---------------------------------------------------------------------------
# walrus compatibility


def _prepare_for_walrus(nc):
    mybir.codegen_inst_isa_subclasses(nc)
    ctr = 0
    for bb in nc.main_func.blocks:
        new = []
        for inst in bb.instructions:
            si = inst.sync_info
            if si is not None and len(si.on_wait) > 1:
                waits = list(si.on_wait)
                for w in waits[:-1]:
                    nop = bass_rust.InstNoOp(
                        name=f"I-wsplit-{ctr}", engine=inst.engine
                    )
                    ctr += 1
                    nop.sync_info = mybir.SyncInfo(on_wait=[w], on_update=[])
                    try:
                        nc.register_instruction(nop, overwrite=True)
                    except Exception:
                        pass
                    new.append(nop)
                si.on_wait = [waits[-1]]
            new.append(inst)
        bb.instructions = new
    return nc


# ---------------------------------------------------------------------------
# device program: psum[32, 1024] = x16[32, 8192] @ codes[8192, 1024]

# staged weight-stream schedule (units of 128-row contraction chunks):
# small chunks first for an early PE start, 1 MiB chunks after for DMA
# efficiency; big pool fully resident so DMA never stalls on reuse
SCHED = [2, 2] + [4] * 15
assert sum(SCHED) == CH


def _build_nc():
    nc = bass.Bass()
    # host layout: wP[p, c, o] = codesT[c*128 + p, o] -> contiguous lines
    wP = nc.dram_tensor("wP", [P, CH * ROWS_PER_CORE], f8,
                        kind="ExternalInput")
    xS = nc.dram_tensor("xS", [P, CH * BATCH], f16, kind="ExternalInput")
    y = nc.dram_tensor("y", [BATCH, ROWS_PER_CORE], f32, kind="ExternalOutput")

    A = mybir.AluOpType

    with tile.TileContext(nc) as tc, ExitStack() as ctx:
        const_pool = ctx.enter_context(tc.tile_pool(name="const", bufs=1))
        pool_m2 = ctx.enter_context(tc.tile_pool(name="wm2", bufs=2))
        pool_m4 = ctx.enter_context(tc.tile_pool(name="wm4", bufs=15))
        psum = ctx.enter_context(tc.tile_pool(name="psum", bufs=1, space="PSUM"))

        # x, host-laid-out as [p, c, b]; two transfers so the first chunk's
        # matmuls gate only on the first half
        xt = const_pool.tile([P, CH, BATCH], f16)
        HX = CH // 2 * BATCH
        nc.sync.dma_start(xt[:, 0:CH // 2, :], xS[:, 0:HX])
        nc.scalar.dma_start(xt[:, CH // 2:CH, :], xS[:, HX:2 * HX])

        ps0 = psum.tile([BATCH, 512], f32, tag="ps0")
        ps1 = psum.tile([BATCH, 512], f32, tag="ps1")

        c = 0
        for k, w in enumerate(SCHED):
            pool = {2: pool_m2, 4: pool_m4}[w]
            wt = pool.tile([P, w, ROWS_PER_CORE], f8)
            eng = nc.sync if k % 2 == 0 else nc.scalar
            eng.dma_start(
                wt[:],
                wP[:, c * ROWS_PER_CORE:(c + w) * ROWS_PER_CORE],
            )
            last = (k == len(SCHED) - 1)
            if last:
                # finish ps0 first so its copy overlaps ps1's last matmuls
                for j in range(w):
                    cc = c + j
                    nc.tensor.matmul(ps0[:], xt[:, cc, :], wt[:, j, 0:512],
                                     start=(cc == 0), stop=(cc == CH - 1))
                for j in range(w):
                    cc = c + j
                    nc.tensor.matmul(ps1[:], xt[:, cc, :], wt[:, j, 512:1024],
                                     start=(cc == 0), stop=(cc == CH - 1))
                c += w
            else:
                for j in range(w):
                    st, sp = (c == 0), False
                    nc.tensor.matmul(ps0[:], xt[:, c, :], wt[:, j, 0:512],
                                     start=st, stop=sp)
                    nc.tensor.matmul(ps1[:], xt[:, c, :], wt[:, j, 512:1024],
                                     start=st, stop=sp)
                    c += 1

        opool = ctx.enter_context(tc.tile_pool(name="o", bufs=1))
        ot = opool.tile([BATCH, ROWS_PER_CORE], f32)
        nc.scalar.copy(ot[:, 0:512], ps0[:])
        nc.sync.dma_start(y[:, 0:512], ot[:, 0:512])
        nc.vector.tensor_scalar(ot[:, 512:768], ps1[:, 0:256], 0.0, None,
                                A.add)
        nc.scalar.copy(ot[:, 768:1024], ps1[:, 256:512])
        nc.scalar.dma_start(y[:, 512:1024], ot[:, 512:1024])

    _prepare_for_walrus(nc)
    return nc


_NC_CACHE = None


def _get_nc():
    global _NC_CACHE
    if _NC_CACHE is None:
        _NC_CACHE = _build_nc()
    return _NC_CACHE


# ---------------------------------------------------------------------------
# host precompute: reference weight transform + e3m4 encoding


def _host_wsim(weight):
    w = np.ascontiguousarray(weight, dtype=np.float32)
    n = w.size
    k_lo = int(n * OUTLIER_FRACTION / 2)
    k_hi = int(n * (1.0 - OUTLIER_FRACTION / 2))
    part = np.partition(w.reshape(-1), [k_lo - 1, k_hi - 1])
    lo = np.float32(part[k_lo - 1])
    hi = np.float32(part[k_hi - 1])
    mask = (w < lo) | (w > hi)
    keep = ~mask
    bscale = np.float32(
        np.sum(np.abs(w) * keep, dtype=np.float32)
        / np.sum(keep, dtype=np.float32)
    )
    # per-row asymmetric 8-bit fake quant, f32 op-for-op like the reference
    w_min = w.min(1, keepdims=True).astype(np.float32)
    w_max = w.max(1, keepdims=True).astype(np.float32)
    rng = (w_max - w_min).astype(np.float32)
    zp = np.round(w_min - np.float32(128.0) * rng / np.float32(255.0)).astype(
        np.float32)
    q = (w - zp).astype(np.float32)
    q = (q * np.float32(255.0)).astype(np.float32)
    q = (q / rng).astype(np.float32)
    q = np.clip(np.round(q), np.float32(0.0), np.float32(255.0)).astype(
        np.float32)
    w_q = (q * (rng / np.float32(255.0)) + zp).astype(np.float32)
    w_sim = np.where(mask, w_q, np.sign(w_q) * bscale).astype(np.float32)
    return w_sim, bscale


def _snap_down_e3m4(v):
    """Largest e3m4-exact value <= v (v positive)."""
    c = v.astype(ml_dtypes.float8_e3m4)
    cf = c.astype(np.float32)
    bits = c.view(np.uint8)
    bits = np.where(cf > v, bits - 1, bits)
    return bits.view(ml_dtypes.float8_e3m4).astype(np.float32)


def _encode_e3m4(w_sim, bscale):
    M = np.abs(w_sim).max(1)
    nb_t = (np.float32(F8MAX) * bscale / M * np.float32(0.999)).astype(
        np.float32)
    nb = _snap_down_e3m4(nb_t)
    s = (bscale / nb).astype(np.float32)
    codes_f = np.clip(w_sim / s[:, None], -F8MAX, F8MAX)
    codes = codes_f.astype(ml_dtypes.float8_e3m4)
    return codes, s


def _run(inputs, trace=False):
    x, weight, bias = inputs["x"], inputs["weight"], inputs["bias"]
    w_sim, bscale = _host_wsim(weight)
    codes, s = _encode_e3m4(w_sim, bscale)

    x2 = np.ascontiguousarray(x, dtype=np.float32).reshape(BATCH, IN_F)
    # [p, c, b] layout: in-feature i = c*128 + p
    xS = np.ascontiguousarray(
        x2.T.reshape(CH, P, BATCH).transpose(1, 0, 2).reshape(P, CH * BATCH)
    ).astype(np.float16)

    nc = _get_nc()
    in_maps = []
    for c in range(N_CORES):
        sl = slice(c * ROWS_PER_CORE, (c + 1) * ROWS_PER_CORE)
        # [in, out] -> [p, c, o] with in = c*128 + p
        cT = codes[sl].T.reshape(CH, P, ROWS_PER_CORE)
        wPc = np.ascontiguousarray(cT.transpose(1, 0, 2)).reshape(
            P, CH * ROWS_PER_CORE)
        in_maps.append({
            "wP": wPc,
            "xS": xS,
        })
    res = run_bass_kernel_spmd(
        nc, in_maps, core_ids=list(range(N_CORES)), trace=trace
    )
    ys = np.concatenate([r["y"] for r in res.results], axis=1)  # [32, 8192]
    out = (ys * s[None, :] + np.asarray(bias, np.float32)[None, :]).reshape(
        BATCH, 1, OUT_F).astype(np.float32)
    return out, res


def kernel(**inputs):
    out, _ = _run(inputs, trace=False)
    return out
